# revision 10
# baseline (speedup 1.0000x reference)
"""BiLSTM-CRF Trainium2 kernel.

Strategy (8 NeuronCores, SPMD single program):
- Even cores run the forward-direction LSTM chains, odd cores run the
  backward chains on time-reversed inputs (host reverses the sentence),
  so one instruction stream serves both directions. 4x pair redundancy.
- The sequential LSTM recurrence is replaced by a Picard fixed-point
  iteration: each iteration computes all 2048 timesteps' gates with one
  batched matmul against the previous iterate's hidden states, solves the
  cell-state recurrence exactly with the hardware scan instruction
  (tensor_tensor_scan), and applies the output gate. The iteration is a
  contraction (rate ~0.57/iter for this data scale); K iterations reach
  the bf16 noise floor.
- Layers are bridged with pair-wise masked AllReduce exchanges (slot0 =
  even core's h, slot1 = odd core's h), with time-flips applied via
  host-supplied 0/1 masks so the program stays parity-independent.
- The CRF Viterbi runs as two exact 1024-step max-plus scans (forward
  deltas and backward suffix scores) interleaved on every core; each step
  is 3 accumulating PE transposes (broadcast of prev state, of the feats
  column, and the transition matrix) + 1 vector max-reduce.
- The host performs the final backtrace from the two delta tables
  (O(L*T) numpy work).
"""

import numpy as np
import ml_dtypes
from contextlib import ExitStack

L, E, H, T = 2048, 512, 512, 64
NEG = -1000.0
K0 = 13  # Picard iterations, layer 0
K1 = 13  # Picard iterations, layer 1
NCH = 4  # time chunks of 512
BF = ml_dtypes.bfloat16

_CACHE = {}


def _build_program():
    import concourse.bacc as bacc
    import concourse.mybir as mybir
    import concourse.tile as tile
    from concourse.masks import make_identity

    F32 = mybir.dt.float32
    BF16 = mybir.dt.bfloat16
    AF = mybir.ActivationFunctionType
    OP = mybir.AluOpType

    nc = bacc.Bacc(None, num_devices=8)

    # ---- I/O ----
    xT_in = nc.dram_tensor("xT", [E, L], BF16, kind="ExternalInput")
    wih0_in = nc.dram_tensor("wih0T", [E, 4 * H], BF16, kind="ExternalInput")
    whh0_in = nc.dram_tensor("whh0T", [H, 4 * H], BF16, kind="ExternalInput")
    b0_in = nc.dram_tensor("b0", [128, 16], F32, kind="ExternalInput")
    wih1_in = nc.dram_tensor("wih1T", [2 * H, 4 * H], BF16, kind="ExternalInput")
    whh1_in = nc.dram_tensor("whh1T", [H, 4 * H], BF16, kind="ExternalInput")
    b1_in = nc.dram_tensor("b1", [128, 16], F32, kind="ExternalInput")
    wtag_in = nc.dram_tensor("wtagT", [2 * H, T], BF16, kind="ExternalInput")
    btag_e_in = nc.dram_tensor("btag_e", [T, 1], F32, kind="ExternalInput")
    btag_o_in = nc.dram_tensor("btag_o", [T, 1], F32, kind="ExternalInput")
    AfT_in = nc.dram_tensor("AfT", [T, T], F32, kind="ExternalInput")  # A.T (fwd chain)
    AbT_in = nc.dram_tensor("AbT", [T, T], F32, kind="ExternalInput")  # A   (bwd chain)
    vf_in = nc.dram_tensor("vf", [T, 1], F32, kind="ExternalInput")
    vb_in = nc.dram_tensor("vb", [T, 1], F32, kind="ExternalInput")
    me_in = nc.dram_tensor("me", [128, 1], F32, kind="ExternalInput")
    mo_in = nc.dram_tensor("mo", [128, 1], F32, kind="ExternalInput")

    Df_out = nc.dram_tensor("Df", [T, 1024], F32, kind="ExternalOutput")
    Db_out = nc.dram_tensor("Db", [T, 1024], F32, kind="ExternalOutput")
    feats_out = nc.dram_tensor("featsT", [T, L], F32, kind="ExternalOutput")

    cc1_in = nc.dram_tensor("cc1_in", [2, H, L], BF16)
    cc1_out = nc.dram_tensor("cc1_out", [2, H, L], BF16)
    cc2_in = nc.dram_tensor("cc2_in", [2, H, L], BF16)
    cc2_out = nc.dram_tensor("cc2_out", [2, H, L], BF16)
    cc3_in = nc.dram_tensor("cc3_in", [2, T, L], F32)
    cc3_out = nc.dram_tensor("cc3_out", [2, T, L], F32)

    PAIRS = [[0, 1], [2, 3], [4, 5], [6, 7]]

    with tile.TileContext(nc) as tc, ExitStack() as top:
        const = top.enter_context(tc.tile_pool(name="const", bufs=1))
        xin_p = top.enter_context(tc.tile_pool(name="xin", bufs=8))
        xg_p = top.enter_context(tc.tile_pool(name="xg", bufs=16))
        whh_p = top.enter_context(tc.tile_pool(name="whh", bufs=4))
        wm_p = top.enter_context(tc.tile_pool(name="wm", bufs=2))
        h_p = top.enter_context(tc.tile_pool(name="h", bufs=8))
        work_p = top.enter_context(tc.tile_pool(name="work", bufs=2))
        c_p = top.enter_context(tc.tile_pool(name="cst", bufs=6))
        ex_p = top.enter_context(tc.tile_pool(name="ex", bufs=2))

        # constants
        identb = const.tile([128, 128], BF16, tag="identb")
        make_identity(nc, identb[:])
        identf = const.tile([T, T], F32, tag="identf")
        make_identity(nc, identf[:])
        b0t = const.tile([128, 16], F32, tag="b0t")
        nc.sync.dma_start(b0t[:], b0_in[:])
        b1t = const.tile([128, 16], F32, tag="b1t")
        nc.sync.dma_start(b1t[:], b1_in[:])
        btag_e = const.tile([T, 1], F32, tag="btag_e")
        nc.sync.dma_start(btag_e[:], btag_e_in[:])
        btag_o = const.tile([T, 1], F32, tag="btag_o")
        nc.sync.dma_start(btag_o[:], btag_o_in[:])
        AfTt = const.tile([T, T], F32, tag="AfTt")
        nc.sync.dma_start(AfTt[:], AfT_in[:])
        AbTt = const.tile([T, T], F32, tag="AbTt")
        nc.sync.dma_start(AbTt[:], AbT_in[:])
        vft = const.tile([T, 1], F32, tag="vft")
        nc.sync.dma_start(vft[:], vf_in[:])
        vbt = const.tile([T, 1], F32, tag="vbt")
        nc.sync.dma_start(vbt[:], vb_in[:])
        met = const.tile([128, 1], F32, tag="met")
        nc.sync.dma_start(met[:], me_in[:])
        mot = const.tile([128, 1], F32, tag="mot")
        nc.sync.dma_start(mot[:], mo_in[:])
        FT = const.tile([T, L], F32, tag="FT")       # true-order feats
        Dft = const.tile([T, 1024], F32, tag="Dft")  # fwd m-values
        Dbt = const.tile([T, 1024], F32, tag="Dbt")  # bwd m-values

        def run_layer(ctx, xin, kin, wih_dram, whh_dram, bt, K):
            """xin: list of kin SBUF tiles [128, L] bf16 (time-aligned inputs).
            Returns list of 4 H-tiles [128, L+2] bf16 (h_t at col 2+t)."""
            ps_pool = ctx.enter_context(tc.tile_pool(name="ps", bufs=2, space="PSUM"))

            # XG precompute: XG[m][:, c*512:+512] = wih.T[:, m-tile] @ xin + b
            xg = []
            for m in range(16):
                wm = wm_p.tile([128, kin * 128], BF16, tag="wm")
                nc.sync.dma_start(
                    wm[:].rearrange("p (k m) -> p k m", k=kin),
                    wih_dram[:, m * 128:(m + 1) * 128]
                    .rearrange("(k p) m -> p k m", p=128),
                )
                xgm = xg_p.tile([128, L], BF16, tag="xg")
                for c in range(NCH):
                    ps = ps_pool.tile([128, 2048], mybir.dt.float32, tag="gates")
                    for k in range(kin):
                        nc.tensor.matmul(
                            ps[:, 0:512],
                            wm[:, k * 128:(k + 1) * 128],
                            xin[k][:, c * 512:(c + 1) * 512],
                            start=(k == 0), stop=(k == kin - 1),
                        )
                    nc.scalar.activation(
                        xgm[:, c * 512:(c + 1) * 512], ps[:, 0:512],
                        AF.Identity, bias=bt[:, m:m + 1], scale=1.0,
                    )
                xg.append(xgm)

            # load whh resident: whh.T is [H, 4H]; tile k holds rows k*128..
            whh = []
            for k in range(4):
                wt = whh_p.tile([128, 4 * H], BF16, tag="whh")
                nc.sync.dma_start(wt[:], whh_dram[k * 128:(k + 1) * 128, :])
                whh.append(wt)

            hbufs = [[h_p.tile([128, L + 2], BF16, tag="h", name=f"hb{b}_{g}")
                      for g in range(4)] for b in range(2)]
            for b in range(2):
                for g in range(4):
                    nc.vector.memset(hbufs[b][g][:], 0.0)

            # gate col layout in psum: i | f | o | g
            m_of = lambda part, g: [g, 4 + g, 12 + g, 8 + g][part]

            for it in range(K):
                hold = hbufs[it % 2]
                hnew = hbufs[1 - it % 2]
                clast = [None] * 4
                for c in range(NCH):
                    for g in range(4):
                        ps = ps_pool.tile([128, 2048], mybir.dt.float32, tag="gates")
                        for part in range(4):
                            m = m_of(part, g)
                            sl = slice(part * 512, part * 512 + 512)
                            for k in range(4):
                                nc.tensor.matmul(
                                    ps[:, sl],
                                    whh[k][:, m * 128:(m + 1) * 128],
                                    hold[k][:, 1 + c * 512: 1 + c * 512 + 512],
                                    start=(k == 0), stop=False,
                                )
                            nc.tensor.matmul(
                                ps[:, sl], identb[:],
                                xg[m][:, c * 512:(c + 1) * 512],
                                start=False, stop=True,
                            )
                        sig = work_p.tile([128, 1536], BF16, tag="sig")
                        nc.scalar.activation(sig[:], ps[:, 0:1536], AF.Sigmoid)
                        tg = work_p.tile([128, 512], BF16, tag="tg")
                        nc.scalar.activation(tg[:], ps[:, 1536:2048], AF.Tanh)
                        u = work_p.tile([128, 512], BF16, tag="u")
                        nc.vector.tensor_mul(u[:], sig[:, 0:512], tg[:])
                        cst = c_p.tile([128, 512], BF16, tag="cst")
                        init = 0.0 if c == 0 else clast[g][:, 511:512]
                        nc.vector.tensor_tensor_scan(
                            out=cst[:], data0=sig[:, 512:1024], data1=u[:],
                            initial=init, op0=OP.mult, op1=OP.add,
                        )
                        clast[g] = cst
                        tct = work_p.tile([128, 512], BF16, tag="tc")
                        nc.scalar.activation(tct[:], cst[:], AF.Tanh)
                        nc.vector.tensor_mul(
                            hnew[g][:, 2 + c * 512: 2 + c * 512 + 512],
                            sig[:, 1024:1536], tct[:],
                        )
            return hbufs[K % 2]

        def exchange(ctx, h4, cc_i, cc_o):
            """Masked pair AllReduce of the 4 h-tiles; returns 8 xin tiles
            [fwd-block(4), bwd-block(4)] in local time order."""
            for k in range(4):
                s0 = ex_p.tile([128, L], BF16, tag="exs", bufs=2, name=f"exs0_{k}")
                nc.vector.tensor_scalar_mul(s0[:], h4[k][:, 2:2 + L], met[:, 0:1])
                nc.sync.dma_start(cc_i[0, k * 128:(k + 1) * 128, :], s0[:])
                s1 = ex_p.tile([128, L], BF16, tag="exs", bufs=2, name=f"exs1_{k}")
                nc.vector.tensor_scalar_mul(s1[:], h4[k][:, 2:2 + L], mot[:, 0:1])
                nc.sync.dma_start(cc_i[1, k * 128:(k + 1) * 128, :], s1[:])
            nc.gpsimd.collective_compute(
                "AllReduce", mybir.AluOpType.add, replica_groups=PAIRS,
                ins=[cc_i[:]], outs=[cc_o[:]],
            )
            xs = []
            for blk in range(2):          # 0: fwd block, 1: bwd block
                mown, mother = (met, mot) if blk == 0 else (mot, met)
                for k in range(4):
                    gsl = ex_p.tile([128, L], BF16, tag="exg")
                    nc.sync.dma_start(gsl[:], cc_o[blk, k * 128:(k + 1) * 128, :])
                    xt = xin_p.tile([128, L], BF16, tag="xin")
                    nc.vector.tensor_scalar_mul(xt[:], gsl[:], mown[:, 0:1])
                    nc.vector.scalar_tensor_tensor(
                        out=xt[:], in0=gsl[:, ::-1], scalar=mother[:, 0:1],
                        in1=xt[:], op0=OP.mult, op1=OP.add,
                    )
                    xs.append(xt)
            return xs

        # ---------------- layer 0 ----------------
        with ExitStack() as ctx0:
            xt0 = []
            for k in range(4):
                x = xin_p.tile([128, L], BF16, tag="xin")
                nc.sync.dma_start(x[:], xT_in[k * 128:(k + 1) * 128, :])
                xt0.append(x)
            h0 = run_layer(ctx0, xt0, 4, wih0_in, whh0_in, b0t, K0)
            x1 = exchange(ctx0, h0, cc1_in, cc1_out)

        # ---------------- layer 1 ----------------
        with ExitStack() as ctx1:
            h1 = run_layer(ctx1, x1, 8, wih1_in, whh1_in, b1t, K1)
            x2 = exchange(ctx1, h1, cc2_in, cc2_out)

        # ---------------- feats ----------------
        with ExitStack() as ctxf:
            ps_pool = ctxf.enter_context(tc.tile_pool(name="psf", bufs=2, space="PSUM"))
            wtag = const.tile([128, 8 * T], BF16, tag="wtag")
            nc.sync.dma_start(
                wtag[:].rearrange("p (k m) -> p k m", k=8),
                wtag_in.rearrange("(k p) m -> p k m", p=128))
            for c in range(NCH):
                ps = ps_pool.tile([T, 512], mybir.dt.float32, tag="fps")
                for k in range(8):
                    nc.tensor.matmul(
                        ps[:], wtag[:, k * T:(k + 1) * T],
                        x2[k][:, c * 512:(c + 1) * 512],
                        start=(k == 0), stop=(k == 7),
                    )
                # masked feats: even cores contribute slot0, odd slot1
                fe = ex_p.tile([T, 512], F32, tag="fex", bufs=2, name=f"fe_{c}")
                nc.scalar.activation(fe[:], ps[:], AF.Tanh,
                                     bias=btag_e[:, 0:1], scale=met[0:T, 0:1])
                nc.sync.dma_start(cc3_in[0, :, c * 512:(c + 1) * 512], fe[:])
                fo = ex_p.tile([T, 512], F32, tag="fex", bufs=2, name=f"fo_{c}")
                nc.scalar.activation(fo[:], ps[:], AF.Tanh,
                                     bias=btag_o[:, 0:1], scale=mot[0:T, 0:1])
                nc.sync.dma_start(cc3_in[1, :, c * 512:(c + 1) * 512], fo[:])
            nc.gpsimd.collective_compute(
                "AllReduce", mybir.AluOpType.add, replica_groups=PAIRS,
                ins=[cc3_in[:]], outs=[cc3_out[:]],
            )
            nc.sync.dma_start(FT[:], cc3_out[0])
            nc.sync.dma_start(feats_out[:], FT[:])

        # ---------------- viterbi: fwd deltas + bwd suffix, interleaved ----
        with ExitStack() as ctxv:
            vps = ctxv.enter_context(tc.tile_pool(name="vps", bufs=8, space="PSUM"))
            for j in range(1024):
                for chain in range(2):
                    At = AfTt if chain == 0 else AbTt
                    Dt = Dft if chain == 0 else Dbt
                    init = vft if chain == 0 else vbt
                    fcol = (j - 1) if chain == 0 else (2048 - j)
                    ps = vps.tile([T, T], mybir.dt.float32, tag="vps")
                    prev = init[:, 0:1] if j == 0 else Dt[:, j - 1:j]
                    nc.tensor.matmul(ps[:], prev.to_broadcast([T, T]), identf[:],
                                     is_transpose=True, start=True, stop=False)
                    if j > 0:
                        nc.tensor.matmul(ps[:], FT[:, fcol:fcol + 1].to_broadcast([T, T]),
                                         identf[:], is_transpose=True,
                                         start=False, stop=False)
                    nc.tensor.matmul(ps[:], At[:], identf[:],
                                     is_transpose=True, start=False, stop=True)
                    nc.vector.tensor_reduce(
                        out=Dt[:, j:j + 1], in_=ps[:],
                        axis=mybir.AxisListType.X, op=mybir.AluOpType.max,
                    )
            nc.sync.dma_start(Df_out[:], Dft[:])
            nc.sync.dma_start(Db_out[:], Dbt[:])

    nc.compile()
    return nc


def _prep_inputs(sentence, emb, w_ih0, w_hh0, b_ih0, b_hh0,
                 w_ih1, w_hh1, b_ih1, b_hh1, W_tag, b_tag, A):
    A = np.asarray(A, np.float32)
    in_maps = []
    for c in range(8):
        d = c % 2  # 0: fwd, 1: bwd
        sent = np.asarray(sentence)
        sent_loc = sent if d == 0 else sent[::-1]
        X = np.asarray(emb)[sent_loc]                      # [L, E]
        m = {
            "xT": np.ascontiguousarray(X.T).astype(BF),
            "wih0T": np.ascontiguousarray(np.asarray(w_ih0)[d].T).astype(BF),
            "whh0T": np.ascontiguousarray(np.asarray(w_hh0)[d].T).astype(BF),
            "b0": np.ascontiguousarray(
                (np.asarray(b_ih0)[d] + np.asarray(b_hh0)[d]).reshape(16, 128).T
            ).astype(np.float32),
            "wih1T": np.ascontiguousarray(np.asarray(w_ih1)[d].T).astype(BF),
            "whh1T": np.ascontiguousarray(np.asarray(w_hh1)[d].T).astype(BF),
            "b1": np.ascontiguousarray(
                (np.asarray(b_ih1)[d] + np.asarray(b_hh1)[d]).reshape(16, 128).T
            ).astype(np.float32),
            "wtagT": np.ascontiguousarray(np.asarray(W_tag).T).astype(BF),
            "btag_e": (np.asarray(b_tag, np.float32) * (1.0 - d)).reshape(T, 1).copy(),
            "btag_o": (np.asarray(b_tag, np.float32) * float(d)).reshape(T, 1).copy(),
            "AfT": np.ascontiguousarray(A.T),
            "AbT": np.ascontiguousarray(A),
            "vf": np.where(np.arange(T) == 0, 0.0, NEG).astype(np.float32).reshape(T, 1),
            "vb": np.where(np.arange(T) == T - 1, 0.0, NEG).astype(np.float32).reshape(T, 1),
            "me": np.full((128, 1), 1.0 - d, np.float32),
            "mo": np.full((128, 1), float(d), np.float32),
        }
        in_maps.append(m)
    return in_maps


def _host_finish(Df_m, Db_m, featsT, A):
    """Backtrace from device delta tables. Df_m/Db_m are m-values (pre-f-add)."""
    A = np.asarray(A, np.float32)
    f = featsT  # [T, L]
    delta1023 = Df_m[:, 1023] + f[:, 1023]
    R1024 = Db_m[:, 1023] + f[:, 1024]
    V = R1024[:, None] + A + delta1023[None, :]
    q, p = np.unravel_index(int(np.argmax(V)), V.shape)
    y = np.zeros(L, np.int64)
    y[1023] = p
    y[1024] = q
    for t in range(1022, -1, -1):
        y[t] = int(np.argmax(Df_m[:, t] + f[:, t] + A[y[t + 1]]))
    for t in range(1025, L):
        Rt = Db_m[:, 2047 - t] + f[:, t]
        y[t] = int(np.argmax(A[:, y[t - 1]] + Rt))
    score = np.float32(V[q, p])
    return y.astype(np.int32), score


def kernel(sentence, emb, w_ih0, w_hh0, b_ih0, b_hh0,
           w_ih1, w_hh1, b_ih1, b_hh1, W_tag, b_tag, A):
    from concourse.bass_utils import run_bass_kernel_spmd

    if "nc" not in _CACHE:
        _CACHE["nc"] = _build_program()
    nc = _CACHE["nc"]

    in_maps = _prep_inputs(sentence, emb, w_ih0, w_hh0, b_ih0, b_hh0,
                           w_ih1, w_hh1, b_ih1, b_hh1, W_tag, b_tag, A)
    res = run_bass_kernel_spmd(nc, in_maps, core_ids=list(range(8)), trace=False)
    r0 = res.results[0]
    _CACHE["last_featsT"] = np.asarray(r0["featsT"], np.float32)
    path, score = _host_finish(
        np.asarray(r0["Df"], np.float32),
        np.asarray(r0["Db"], np.float32),
        np.asarray(r0["featsT"], np.float32),
        np.asarray(A, np.float32),
    )
    return path, np.float32(score)


# revision 11
# speedup vs baseline: 2.7360x; 2.7360x over previous
"""BiLSTM-CRF Trainium2 kernel.

Strategy (8 NeuronCores, SPMD single program):
- Even cores run the forward-direction LSTM chains, odd cores run the
  backward chains on time-reversed inputs (host reverses the sentence),
  so one instruction stream serves both directions. 4x pair redundancy.
- The sequential LSTM recurrence is replaced by a Picard fixed-point
  iteration: each iteration computes all 2048 timesteps' gates with one
  batched matmul against the previous iterate's hidden states, solves the
  cell-state recurrence exactly with the hardware scan instruction
  (tensor_tensor_scan), and applies the output gate. The iteration is a
  contraction (rate ~0.57/iter for this data scale); K iterations reach
  the bf16 noise floor.
- Layers are bridged with pair-wise masked AllReduce exchanges (slot0 =
  even core's h, slot1 = odd core's h), with time-flips applied via
  host-supplied 0/1 masks so the program stays parity-independent.
- The CRF Viterbi runs as two exact 1024-step max-plus scans (forward
  deltas and backward suffix scores) interleaved on every core; each step
  is 3 accumulating PE transposes (broadcast of prev state, of the feats
  column, and the transition matrix) + 1 vector max-reduce.
- The host performs the final backtrace from the two delta tables
  (O(L*T) numpy work).
"""

import numpy as np
import ml_dtypes
from contextlib import ExitStack

L, E, H, T = 2048, 512, 512, 64
NEG = -1000.0
K0 = 13  # Picard iterations, layer 0
K1 = 13  # Picard iterations, layer 1
NCH = 4  # time chunks of 512
BF = ml_dtypes.bfloat16

_CACHE = {}


def _build_program():
    import concourse.bacc as bacc
    import concourse.mybir as mybir
    import concourse.tile as tile
    from concourse.masks import make_identity

    F32 = mybir.dt.float32
    BF16 = mybir.dt.bfloat16
    AF = mybir.ActivationFunctionType
    OP = mybir.AluOpType

    nc = bacc.Bacc(None, num_devices=8)

    # ---- I/O ----
    xT_in = nc.dram_tensor("xT", [E, L], BF16, kind="ExternalInput")
    wih0_in = nc.dram_tensor("wih0T", [E, 4 * H], BF16, kind="ExternalInput")
    whh0_in = nc.dram_tensor("whh0T", [H, 4 * H], BF16, kind="ExternalInput")
    b0_in = nc.dram_tensor("b0", [128, 16], F32, kind="ExternalInput")
    wih1_in = nc.dram_tensor("wih1T", [2 * H, 4 * H], BF16, kind="ExternalInput")
    whh1_in = nc.dram_tensor("whh1T", [H, 4 * H], BF16, kind="ExternalInput")
    b1_in = nc.dram_tensor("b1", [128, 16], F32, kind="ExternalInput")
    wtag_in = nc.dram_tensor("wtagT", [2 * H, T], BF16, kind="ExternalInput")
    btag_e_in = nc.dram_tensor("btag_e", [T, 1], F32, kind="ExternalInput")
    btag_o_in = nc.dram_tensor("btag_o", [T, 1], F32, kind="ExternalInput")
    AfT_in = nc.dram_tensor("AfT", [T, T], F32, kind="ExternalInput")  # A.T (fwd chain)
    AbT_in = nc.dram_tensor("AbT", [T, T], F32, kind="ExternalInput")  # A   (bwd chain)
    vf_in = nc.dram_tensor("vf", [T, 1], F32, kind="ExternalInput")
    vb_in = nc.dram_tensor("vb", [T, 1], F32, kind="ExternalInput")
    me_in = nc.dram_tensor("me", [128, 1], F32, kind="ExternalInput")
    mo_in = nc.dram_tensor("mo", [128, 1], F32, kind="ExternalInput")

    Df_out = nc.dram_tensor("Df", [T, 1024], F32, kind="ExternalOutput")
    Db_out = nc.dram_tensor("Db", [T, 1024], F32, kind="ExternalOutput")
    feats_out = nc.dram_tensor("featsT", [T, L], F32, kind="ExternalOutput")

    cc1_in = nc.dram_tensor("cc1_in", [2, H, L], BF16)
    cc1_out = nc.dram_tensor("cc1_out", [2, H, L], BF16)
    cc2_in = nc.dram_tensor("cc2_in", [2, H, L], BF16)
    cc2_out = nc.dram_tensor("cc2_out", [2, H, L], BF16)
    cc3_in = nc.dram_tensor("cc3_in", [2, T, L], F32)
    cc3_out = nc.dram_tensor("cc3_out", [2, T, L], F32)

    PAIRS = [[0, 1], [2, 3], [4, 5], [6, 7]]

    with tile.TileContext(nc) as tc, ExitStack() as top:
        const = top.enter_context(tc.tile_pool(name="const", bufs=1))
        xin_p = top.enter_context(tc.tile_pool(name="xin", bufs=8))
        xg_p = top.enter_context(tc.tile_pool(name="xg", bufs=16))
        whh_p = top.enter_context(tc.tile_pool(name="whh", bufs=4))
        wm_p = top.enter_context(tc.tile_pool(name="wm", bufs=2))
        h_p = top.enter_context(tc.tile_pool(name="h", bufs=8))
        work_p = top.enter_context(tc.tile_pool(name="work", bufs=2))
        c_p = top.enter_context(tc.tile_pool(name="cst", bufs=6))
        ex_p = top.enter_context(tc.tile_pool(name="ex", bufs=2))

        # constants
        identb = const.tile([128, 128], BF16, tag="identb")
        make_identity(nc, identb[:])
        identf = const.tile([T, T], F32, tag="identf")
        make_identity(nc, identf[:])
        b0t = const.tile([128, 16], F32, tag="b0t")
        nc.sync.dma_start(b0t[:], b0_in[:])
        b1t = const.tile([128, 16], F32, tag="b1t")
        nc.sync.dma_start(b1t[:], b1_in[:])
        btag_e = const.tile([T, 1], F32, tag="btag_e")
        nc.sync.dma_start(btag_e[:], btag_e_in[:])
        btag_o = const.tile([T, 1], F32, tag="btag_o")
        nc.sync.dma_start(btag_o[:], btag_o_in[:])
        AfTt = const.tile([T, T], F32, tag="AfTt")
        nc.sync.dma_start(AfTt[:], AfT_in[:])
        AbTt = const.tile([T, T], F32, tag="AbTt")
        nc.sync.dma_start(AbTt[:], AbT_in[:])
        vft = const.tile([T, 1], F32, tag="vft")
        nc.sync.dma_start(vft[:], vf_in[:])
        vbt = const.tile([T, 1], F32, tag="vbt")
        nc.sync.dma_start(vbt[:], vb_in[:])
        met = const.tile([128, 1], F32, tag="met")
        nc.sync.dma_start(met[:], me_in[:])
        mot = const.tile([128, 1], F32, tag="mot")
        nc.sync.dma_start(mot[:], mo_in[:])
        FT = const.tile([T, L], F32, tag="FT")       # true-order feats
        Dft = const.tile([T, 1024], F32, tag="Dft")  # fwd m-values
        Dbt = const.tile([T, 1024], F32, tag="Dbt")  # bwd m-values

        def run_layer(ctx, xin, kin, wih_dram, whh_dram, bt, K):
            """xin: list of kin SBUF tiles [128, L] bf16 (time-aligned inputs).
            Returns list of 4 H-tiles [128, L+2] bf16 (h_t at col 2+t)."""
            ps_pool = ctx.enter_context(tc.tile_pool(name="ps", bufs=2, space="PSUM"))

            # XG precompute: XG[m][:, c*512:+512] = wih.T[:, m-tile] @ xin + b
            xg = []
            for m in range(16):
                wm = wm_p.tile([128, kin * 128], BF16, tag="wm")
                nc.sync.dma_start(
                    wm[:].rearrange("p (k m) -> p k m", k=kin),
                    wih_dram[:, m * 128:(m + 1) * 128]
                    .rearrange("(k p) m -> p k m", p=128),
                )
                xgm = xg_p.tile([128, L], BF16, tag="xg")
                for c in range(NCH):
                    ps = ps_pool.tile([128, 2048], mybir.dt.float32, tag="gates")
                    for k in range(kin):
                        nc.tensor.matmul(
                            ps[:, 0:512],
                            wm[:, k * 128:(k + 1) * 128],
                            xin[k][:, c * 512:(c + 1) * 512],
                            start=(k == 0), stop=(k == kin - 1),
                        )
                    nc.scalar.activation(
                        xgm[:, c * 512:(c + 1) * 512], ps[:, 0:512],
                        AF.Identity, bias=bt[:, m:m + 1], scale=1.0,
                    )
                xg.append(xgm)

            # load whh resident: whh.T is [H, 4H]; tile k holds rows k*128..
            whh = []
            for k in range(4):
                wt = whh_p.tile([128, 4 * H], BF16, tag="whh")
                nc.sync.dma_start(wt[:], whh_dram[k * 128:(k + 1) * 128, :])
                whh.append(wt)

            hbufs = [[h_p.tile([128, L + 2], BF16, tag="h", name=f"hb{b}_{g}")
                      for g in range(4)] for b in range(2)]
            for b in range(2):
                for g in range(4):
                    nc.vector.memset(hbufs[b][g][:], 0.0)

            # gate col layout in psum: i | f | o | g
            m_of = lambda part, g: [g, 4 + g, 12 + g, 8 + g][part]

            for it in range(K):
                hold = hbufs[it % 2]
                hnew = hbufs[1 - it % 2]
                clast = [None] * 4
                for c in range(NCH):
                    for g in range(4):
                        ps = ps_pool.tile([128, 2048], mybir.dt.float32, tag="gates")
                        for part in range(4):
                            m = m_of(part, g)
                            sl = slice(part * 512, part * 512 + 512)
                            for k in range(4):
                                nc.tensor.matmul(
                                    ps[:, sl],
                                    whh[k][:, m * 128:(m + 1) * 128],
                                    hold[k][:, 1 + c * 512: 1 + c * 512 + 512],
                                    start=(k == 0), stop=False,
                                )
                            nc.tensor.matmul(
                                ps[:, sl], identb[:],
                                xg[m][:, c * 512:(c + 1) * 512],
                                start=False, stop=True,
                            )
                        sig = work_p.tile([128, 1536], BF16, tag="sig")
                        nc.scalar.activation(sig[:], ps[:, 0:1536], AF.Sigmoid)
                        tg = work_p.tile([128, 512], BF16, tag="tg")
                        nc.scalar.activation(tg[:], ps[:, 1536:2048], AF.Tanh)
                        u = work_p.tile([128, 512], BF16, tag="u")
                        nc.vector.tensor_mul(u[:], sig[:, 0:512], tg[:])
                        cst = c_p.tile([128, 512], BF16, tag="cst")
                        init = 0.0 if c == 0 else clast[g][:, 511:512]
                        nc.vector.tensor_tensor_scan(
                            out=cst[:], data0=sig[:, 512:1024], data1=u[:],
                            initial=init, op0=OP.mult, op1=OP.add,
                        )
                        clast[g] = cst
                        tct = work_p.tile([128, 512], BF16, tag="tc")
                        nc.scalar.activation(tct[:], cst[:], AF.Tanh)
                        nc.vector.tensor_mul(
                            hnew[g][:, 2 + c * 512: 2 + c * 512 + 512],
                            sig[:, 1024:1536], tct[:],
                        )
            return hbufs[K % 2]

        def exchange(ctx, h4, cc_i, cc_o):
            """Masked pair AllReduce of the 4 h-tiles; returns 8 xin tiles
            [fwd-block(4), bwd-block(4)] in local time order."""
            for k in range(4):
                s0 = ex_p.tile([128, L], BF16, tag="exs", bufs=2, name=f"exs0_{k}")
                nc.vector.tensor_scalar_mul(s0[:], h4[k][:, 2:2 + L], met[:, 0:1])
                nc.sync.dma_start(cc_i[0, k * 128:(k + 1) * 128, :], s0[:])
                s1 = ex_p.tile([128, L], BF16, tag="exs", bufs=2, name=f"exs1_{k}")
                nc.vector.tensor_scalar_mul(s1[:], h4[k][:, 2:2 + L], mot[:, 0:1])
                nc.sync.dma_start(cc_i[1, k * 128:(k + 1) * 128, :], s1[:])
            nc.gpsimd.collective_compute(
                "AllReduce", mybir.AluOpType.add, replica_groups=PAIRS,
                ins=[cc_i[:]], outs=[cc_o[:]],
            )
            xs = []
            for blk in range(2):          # 0: fwd block, 1: bwd block
                mown, mother = (met, mot) if blk == 0 else (mot, met)
                for k in range(4):
                    gsl = ex_p.tile([128, L], BF16, tag="exg")
                    nc.sync.dma_start(gsl[:], cc_o[blk, k * 128:(k + 1) * 128, :])
                    xt = xin_p.tile([128, L], BF16, tag="xin")
                    nc.vector.tensor_scalar_mul(xt[:], gsl[:], mown[:, 0:1])
                    nc.vector.scalar_tensor_tensor(
                        out=xt[:], in0=gsl[:, ::-1], scalar=mother[:, 0:1],
                        in1=xt[:], op0=OP.mult, op1=OP.add,
                    )
                    xs.append(xt)
            return xs

        # ---------------- layer 0 ----------------
        with ExitStack() as ctx0:
            xt0 = []
            for k in range(4):
                x = xin_p.tile([128, L], BF16, tag="xin")
                nc.sync.dma_start(x[:], xT_in[k * 128:(k + 1) * 128, :])
                xt0.append(x)
            h0 = run_layer(ctx0, xt0, 4, wih0_in, whh0_in, b0t, K0)
            x1 = exchange(ctx0, h0, cc1_in, cc1_out)

        # ---------------- layer 1 ----------------
        with ExitStack() as ctx1:
            h1 = run_layer(ctx1, x1, 8, wih1_in, whh1_in, b1t, K1)
            x2 = exchange(ctx1, h1, cc2_in, cc2_out)

        # ---------------- feats ----------------
        with ExitStack() as ctxf:
            ps_pool = ctxf.enter_context(tc.tile_pool(name="psf", bufs=2, space="PSUM"))
            wtag = const.tile([128, 8 * T], BF16, tag="wtag")
            nc.sync.dma_start(
                wtag[:].rearrange("p (k m) -> p k m", k=8),
                wtag_in.rearrange("(k p) m -> p k m", p=128))
            for c in range(NCH):
                ps = ps_pool.tile([T, 512], mybir.dt.float32, tag="fps")
                for k in range(8):
                    nc.tensor.matmul(
                        ps[:], wtag[:, k * T:(k + 1) * T],
                        x2[k][:, c * 512:(c + 1) * 512],
                        start=(k == 0), stop=(k == 7),
                    )
                # masked feats: even cores contribute slot0, odd slot1
                fe = ex_p.tile([T, 512], F32, tag="fex", bufs=2, name=f"fe_{c}")
                nc.scalar.activation(fe[:], ps[:], AF.Tanh,
                                     bias=btag_e[:, 0:1], scale=met[0:T, 0:1])
                nc.sync.dma_start(cc3_in[0, :, c * 512:(c + 1) * 512], fe[:])
                fo = ex_p.tile([T, 512], F32, tag="fex", bufs=2, name=f"fo_{c}")
                nc.scalar.activation(fo[:], ps[:], AF.Tanh,
                                     bias=btag_o[:, 0:1], scale=mot[0:T, 0:1])
                nc.sync.dma_start(cc3_in[1, :, c * 512:(c + 1) * 512], fo[:])
            nc.gpsimd.collective_compute(
                "AllReduce", mybir.AluOpType.add, replica_groups=PAIRS,
                ins=[cc3_in[:]], outs=[cc3_out[:]],
            )
            nc.sync.dma_start(FT[:], cc3_out[0])
            nc.sync.dma_start(feats_out[:], FT[:])

        # ---------------- viterbi: fwd deltas + bwd suffix, interleaved ----
        with ExitStack() as ctxv:
            vps = ctxv.enter_context(tc.tile_pool(name="vps", bufs=8, space="PSUM"))
            for j in range(1024):
                for chain in range(2):
                    At = AfTt if chain == 0 else AbTt
                    Dt = Dft if chain == 0 else Dbt
                    init = vft if chain == 0 else vbt
                    fcol = (j - 1) if chain == 0 else (2048 - j)
                    ps = vps.tile([T, T], mybir.dt.float32, tag="vps")
                    prev = init[:, 0:1] if j == 0 else Dt[:, j - 1:j]
                    nc.tensor.matmul(ps[:], prev.to_broadcast([T, T]), identf[:],
                                     is_transpose=True, start=True, stop=False)
                    if j > 0:
                        nc.tensor.matmul(ps[:], FT[:, fcol:fcol + 1].to_broadcast([T, T]),
                                         identf[:], is_transpose=True,
                                         start=False, stop=False)
                    nc.tensor.matmul(ps[:], At[:], identf[:],
                                     is_transpose=True, start=False, stop=True)
                    nc.vector.tensor_reduce(
                        out=Dt[:, j:j + 1], in_=ps[:],
                        axis=mybir.AxisListType.X, op=mybir.AluOpType.max,
                    )
            nc.sync.dma_start(Df_out[:], Dft[:])
            nc.sync.dma_start(Db_out[:], Dbt[:])

    nc.compile()
    return nc


def _prep_inputs(sentence, emb, w_ih0, w_hh0, b_ih0, b_hh0,
                 w_ih1, w_hh1, b_ih1, b_hh1, W_tag, b_tag, A):
    A = np.asarray(A, np.float32)
    in_maps = []
    for c in range(8):
        d = c % 2  # 0: fwd, 1: bwd
        sent = np.asarray(sentence)
        sent_loc = sent if d == 0 else sent[::-1]
        X = np.asarray(emb)[sent_loc]                      # [L, E]
        m = {
            "xT": np.ascontiguousarray(X.T).astype(BF),
            "wih0T": np.ascontiguousarray(np.asarray(w_ih0)[d].T).astype(BF),
            "whh0T": np.ascontiguousarray(np.asarray(w_hh0)[d].T).astype(BF),
            "b0": np.ascontiguousarray(
                (np.asarray(b_ih0)[d] + np.asarray(b_hh0)[d]).reshape(16, 128).T
            ).astype(np.float32),
            "wih1T": np.ascontiguousarray(np.asarray(w_ih1)[d].T).astype(BF),
            "whh1T": np.ascontiguousarray(np.asarray(w_hh1)[d].T).astype(BF),
            "b1": np.ascontiguousarray(
                (np.asarray(b_ih1)[d] + np.asarray(b_hh1)[d]).reshape(16, 128).T
            ).astype(np.float32),
            "wtagT": np.ascontiguousarray(np.asarray(W_tag).T).astype(BF),
            "btag_e": (np.asarray(b_tag, np.float32) * (1.0 - d)).reshape(T, 1).copy(),
            "btag_o": (np.asarray(b_tag, np.float32) * float(d)).reshape(T, 1).copy(),
            "AfT": np.ascontiguousarray(A.T),
            "AbT": np.ascontiguousarray(A),
            "vf": np.where(np.arange(T) == 0, 0.0, NEG).astype(np.float32).reshape(T, 1),
            "vb": np.where(np.arange(T) == T - 1, 0.0, NEG).astype(np.float32).reshape(T, 1),
            "me": np.full((128, 1), 1.0 - d, np.float32),
            "mo": np.full((128, 1), float(d), np.float32),
        }
        in_maps.append(m)
    return in_maps


def _host_finish(Df_m, Db_m, featsT, A):
    """Backtrace from device delta tables. Df_m/Db_m are m-values (pre-f-add)."""
    A = np.asarray(A, np.float32)
    f = featsT  # [T, L]
    delta1023 = Df_m[:, 1023] + f[:, 1023]
    R1024 = Db_m[:, 1023] + f[:, 1024]
    V = R1024[:, None] + A + delta1023[None, :]
    q, p = np.unravel_index(int(np.argmax(V)), V.shape)
    y = np.zeros(L, np.int64)
    y[1023] = p
    y[1024] = q
    for t in range(1022, -1, -1):
        y[t] = int(np.argmax(Df_m[:, t] + f[:, t] + A[y[t + 1]]))
    for t in range(1025, L):
        Rt = Db_m[:, 2047 - t] + f[:, t]
        y[t] = int(np.argmax(A[:, y[t - 1]] + Rt))
    score = np.float32(V[q, p])
    return y.astype(np.int32), score


def _make_runner(nc):
    """Cached jitted SPMD executor (run_bass_via_pjrt re-jits every call;
    this builds the shard_map jit once and reuses it)."""
    import jax
    import numpy as _np
    import concourse.mybir as mybir
    from concourse import bass2jax
    from jax.experimental.shard_map import shard_map
    from jax.sharding import Mesh, PartitionSpec

    bass2jax.install_neuronx_cc_hook()
    n_cores = 8
    partition_name = nc.partition_id_tensor.name if nc.partition_id_tensor else None
    in_names, out_names, out_avals, zero_outs = [], [], [], []
    for alloc in nc.m.functions[0].allocations:
        if not isinstance(alloc, mybir.MemoryLocationSet):
            continue
        name = alloc.memorylocations[0].name
        if alloc.kind == "ExternalInput":
            if name != partition_name:
                in_names.append(name)
        elif alloc.kind == "ExternalOutput":
            shape = tuple(alloc.tensor_shape)
            dtype = mybir.dt.np(alloc.dtype)
            out_names.append(name)
            out_avals.append(jax.core.ShapedArray(shape, dtype))
            zero_outs.append(_np.zeros(shape, dtype))
    n_params = len(in_names)
    n_outs = len(out_avals)
    all_in_names = list(in_names) + list(out_names)
    if partition_name is not None:
        all_in_names.append(partition_name)
    donate = tuple(range(n_params, n_params + n_outs))

    def _body(*args):
        operands = list(args)
        if partition_name is not None:
            operands.append(bass2jax.partition_id_tensor())
        outs = bass2jax._bass_exec_p.bind(
            *operands,
            out_avals=tuple(out_avals),
            in_names=tuple(all_in_names),
            out_names=tuple(out_names),
            lowering_input_output_aliases=(),
            sim_require_finite=True,
            sim_require_nnan=True,
            nc=nc,
        )
        return tuple(outs)

    devices = jax.devices()[:n_cores]
    mesh = Mesh(_np.asarray(devices), ("core",))
    in_specs = (PartitionSpec("core"),) * (n_params + n_outs)
    out_specs = (PartitionSpec("core"),) * n_outs
    sharded = jax.jit(
        shard_map(_body, mesh=mesh, in_specs=in_specs, out_specs=out_specs,
                  check_rep=False),
        donate_argnums=donate, keep_unused=True,
    )

    def run(in_maps):
        concat_in = [
            _np.concatenate([_np.asarray(in_maps[c][nm]) for c in range(n_cores)], axis=0)
            for nm in in_names
        ]
        concat_zeros = [
            _np.zeros((n_cores * z.shape[0], *z.shape[1:]), z.dtype)
            for z in zero_outs
        ]
        out_arrs = sharded(*concat_in, *concat_zeros)
        core0 = {
            nm: _np.asarray(out_arrs[i]).reshape(n_cores, *out_avals[i].shape)[0]
            for i, nm in enumerate(out_names)
        }
        return core0

    return run


def kernel(sentence, emb, w_ih0, w_hh0, b_ih0, b_hh0,
           w_ih1, w_hh1, b_ih1, b_hh1, W_tag, b_tag, A):
    if "nc" not in _CACHE:
        _CACHE["nc"] = _build_program()
        _CACHE["run"] = _make_runner(_CACHE["nc"])

    in_maps = _prep_inputs(sentence, emb, w_ih0, w_hh0, b_ih0, b_hh0,
                           w_ih1, w_hh1, b_ih1, b_hh1, W_tag, b_tag, A)
    r0 = _CACHE["run"](in_maps)
    _CACHE["last_featsT"] = np.asarray(r0["featsT"], np.float32)
    path, score = _host_finish(
        np.asarray(r0["Df"], np.float32),
        np.asarray(r0["Db"], np.float32),
        np.asarray(r0["featsT"], np.float32),
        np.asarray(A, np.float32),
    )
    return path, np.float32(score)


# revision 12
# speedup vs baseline: 10.9158x; 3.9897x over previous
"""BiLSTM-CRF Trainium2 kernel.

Strategy (8 NeuronCores, SPMD single program):
- Even cores run the forward-direction LSTM chains, odd cores run the
  backward chains on time-reversed inputs (host reverses the sentence),
  so one instruction stream serves both directions. 4x pair redundancy.
- The sequential LSTM recurrence is replaced by a Picard fixed-point
  iteration: each iteration computes all 2048 timesteps' gates with one
  batched matmul against the previous iterate's hidden states, solves the
  cell-state recurrence exactly with the hardware scan instruction
  (tensor_tensor_scan), and applies the output gate. The iteration is a
  contraction (rate ~0.57/iter for this data scale); K iterations reach
  the bf16 noise floor.
- Layers are bridged with pair-wise masked AllReduce exchanges (slot0 =
  even core's h, slot1 = odd core's h), with time-flips applied via
  host-supplied 0/1 masks so the program stays parity-independent.
- The CRF Viterbi runs as two exact 1024-step max-plus scans (forward
  deltas and backward suffix scores) interleaved on every core; each step
  is 3 accumulating PE transposes (broadcast of prev state, of the feats
  column, and the transition matrix) + 1 vector max-reduce.
- The host performs the final backtrace from the two delta tables
  (O(L*T) numpy work).
"""

import numpy as np
import ml_dtypes
from contextlib import ExitStack

L, E, H, T = 2048, 512, 512, 64
NEG = -1000.0
K0 = 13  # Picard iterations, layer 0
K1 = 13  # Picard iterations, layer 1
NCH = 4  # time chunks of 512
BF = ml_dtypes.bfloat16
NCORES = 2

_CACHE = {}


def _build_program():
    import concourse.bacc as bacc
    import concourse.mybir as mybir
    import concourse.tile as tile
    from concourse.masks import make_identity

    F32 = mybir.dt.float32
    BF16 = mybir.dt.bfloat16
    AF = mybir.ActivationFunctionType
    OP = mybir.AluOpType

    nc = bacc.Bacc(None, num_devices=NCORES)

    # ---- I/O ----
    xT_in = nc.dram_tensor("xT", [E, L], BF16, kind="ExternalInput")
    wih0_in = nc.dram_tensor("wih0T", [E, 4 * H], BF16, kind="ExternalInput")
    whh0_in = nc.dram_tensor("whh0T", [H, 4 * H], BF16, kind="ExternalInput")
    b0_in = nc.dram_tensor("b0", [128, 16], F32, kind="ExternalInput")
    wih1_in = nc.dram_tensor("wih1T", [2 * H, 4 * H], BF16, kind="ExternalInput")
    whh1_in = nc.dram_tensor("whh1T", [H, 4 * H], BF16, kind="ExternalInput")
    b1_in = nc.dram_tensor("b1", [128, 16], F32, kind="ExternalInput")
    wtag_in = nc.dram_tensor("wtagT", [2 * H, T], BF16, kind="ExternalInput")
    btag_e_in = nc.dram_tensor("btag_e", [T, 1], F32, kind="ExternalInput")
    btag_o_in = nc.dram_tensor("btag_o", [T, 1], F32, kind="ExternalInput")
    AfT_in = nc.dram_tensor("AfT", [T, T], F32, kind="ExternalInput")  # A.T (fwd chain)
    AbT_in = nc.dram_tensor("AbT", [T, T], F32, kind="ExternalInput")  # A   (bwd chain)
    vf_in = nc.dram_tensor("vf", [T, 1], F32, kind="ExternalInput")
    vb_in = nc.dram_tensor("vb", [T, 1], F32, kind="ExternalInput")
    me_in = nc.dram_tensor("me", [128, 1], F32, kind="ExternalInput")
    mo_in = nc.dram_tensor("mo", [128, 1], F32, kind="ExternalInput")

    Df_out = nc.dram_tensor("Df", [T, 1024], F32, kind="ExternalOutput")
    Db_out = nc.dram_tensor("Db", [T, 1024], F32, kind="ExternalOutput")
    feats_out = nc.dram_tensor("featsT", [T, L], F32, kind="ExternalOutput")

    cc1_in = nc.dram_tensor("cc1_in", [2, H, L], BF16)
    cc1_out = nc.dram_tensor("cc1_out", [2, H, L], BF16)
    cc2_in = nc.dram_tensor("cc2_in", [2, H, L], BF16)
    cc2_out = nc.dram_tensor("cc2_out", [2, H, L], BF16)
    cc3_in = nc.dram_tensor("cc3_in", [2, T, L], F32)
    cc3_out = nc.dram_tensor("cc3_out", [2, T, L], F32)

    PAIRS = [[2 * i, 2 * i + 1] for i in range(NCORES // 2)]

    with tile.TileContext(nc) as tc, ExitStack() as top:
        const = top.enter_context(tc.tile_pool(name="const", bufs=1))
        xin_p = top.enter_context(tc.tile_pool(name="xin", bufs=8))
        xg_p = top.enter_context(tc.tile_pool(name="xg", bufs=16))
        whh_p = top.enter_context(tc.tile_pool(name="whh", bufs=4))
        wm_p = top.enter_context(tc.tile_pool(name="wm", bufs=2))
        h_p = top.enter_context(tc.tile_pool(name="h", bufs=8))
        work_p = top.enter_context(tc.tile_pool(name="work", bufs=2))
        c_p = top.enter_context(tc.tile_pool(name="cst", bufs=6))
        ex_p = top.enter_context(tc.tile_pool(name="ex", bufs=2))

        # constants
        identb = const.tile([128, 128], BF16, tag="identb")
        make_identity(nc, identb[:])
        identf = const.tile([T, T], F32, tag="identf")
        make_identity(nc, identf[:])
        b0t = const.tile([128, 16], F32, tag="b0t")
        nc.sync.dma_start(b0t[:], b0_in[:])
        b1t = const.tile([128, 16], F32, tag="b1t")
        nc.sync.dma_start(b1t[:], b1_in[:])
        btag_e = const.tile([T, 1], F32, tag="btag_e")
        nc.sync.dma_start(btag_e[:], btag_e_in[:])
        btag_o = const.tile([T, 1], F32, tag="btag_o")
        nc.sync.dma_start(btag_o[:], btag_o_in[:])
        AfTt = const.tile([T, T], F32, tag="AfTt")
        nc.sync.dma_start(AfTt[:], AfT_in[:])
        AbTt = const.tile([T, T], F32, tag="AbTt")
        nc.sync.dma_start(AbTt[:], AbT_in[:])
        vft = const.tile([T, 1], F32, tag="vft")
        nc.sync.dma_start(vft[:], vf_in[:])
        vbt = const.tile([T, 1], F32, tag="vbt")
        nc.sync.dma_start(vbt[:], vb_in[:])
        met = const.tile([128, 1], F32, tag="met")
        nc.sync.dma_start(met[:], me_in[:])
        mot = const.tile([128, 1], F32, tag="mot")
        nc.sync.dma_start(mot[:], mo_in[:])
        FT = const.tile([T, L], F32, tag="FT")       # true-order feats
        Dft = const.tile([T, 1024], F32, tag="Dft")  # fwd m-values
        Dbt = const.tile([T, 1024], F32, tag="Dbt")  # bwd m-values

        def run_layer(ctx, xin, kin, wih_dram, whh_dram, bt, K):
            """xin: list of kin SBUF tiles [128, L] bf16 (time-aligned inputs).
            Returns list of 4 H-tiles [128, L+2] bf16 (h_t at col 2+t)."""
            ps_pool = ctx.enter_context(tc.tile_pool(name="ps", bufs=2, space="PSUM"))

            # XG precompute: XG[m][:, c*512:+512] = wih.T[:, m-tile] @ xin + b
            xg = []
            for m in range(16):
                wm = wm_p.tile([128, kin * 128], BF16, tag="wm")
                nc.sync.dma_start(
                    wm[:].rearrange("p (k m) -> p k m", k=kin),
                    wih_dram[:, m * 128:(m + 1) * 128]
                    .rearrange("(k p) m -> p k m", p=128),
                )
                xgm = xg_p.tile([128, L], BF16, tag="xg")
                for c in range(NCH):
                    ps = ps_pool.tile([128, 2048], mybir.dt.float32, tag="gates")
                    for k in range(kin):
                        nc.tensor.matmul(
                            ps[:, 0:512],
                            wm[:, k * 128:(k + 1) * 128],
                            xin[k][:, c * 512:(c + 1) * 512],
                            start=(k == 0), stop=(k == kin - 1),
                        )
                    nc.scalar.activation(
                        xgm[:, c * 512:(c + 1) * 512], ps[:, 0:512],
                        AF.Identity, bias=bt[:, m:m + 1], scale=1.0,
                    )
                xg.append(xgm)

            # load whh resident: whh.T is [H, 4H]; tile k holds rows k*128..
            whh = []
            for k in range(4):
                wt = whh_p.tile([128, 4 * H], BF16, tag="whh")
                nc.sync.dma_start(wt[:], whh_dram[k * 128:(k + 1) * 128, :])
                whh.append(wt)

            hbufs = [[h_p.tile([128, L + 2], BF16, tag="h", name=f"hb{b}_{g}")
                      for g in range(4)] for b in range(2)]
            for b in range(2):
                for g in range(4):
                    nc.vector.memset(hbufs[b][g][:], 0.0)

            # gate col layout in psum: i | f | o | g
            m_of = lambda part, g: [g, 4 + g, 12 + g, 8 + g][part]

            for it in range(K):
                hold = hbufs[it % 2]
                hnew = hbufs[1 - it % 2]
                clast = [None] * 4
                for c in range(NCH):
                    for g in range(4):
                        ps = ps_pool.tile([128, 2048], mybir.dt.float32, tag="gates")
                        for part in range(4):
                            m = m_of(part, g)
                            sl = slice(part * 512, part * 512 + 512)
                            for k in range(4):
                                nc.tensor.matmul(
                                    ps[:, sl],
                                    whh[k][:, m * 128:(m + 1) * 128],
                                    hold[k][:, 1 + c * 512: 1 + c * 512 + 512],
                                    start=(k == 0), stop=False,
                                )
                            nc.tensor.matmul(
                                ps[:, sl], identb[:],
                                xg[m][:, c * 512:(c + 1) * 512],
                                start=False, stop=True,
                            )
                        sig = work_p.tile([128, 1536], BF16, tag="sig")
                        nc.scalar.activation(sig[:], ps[:, 0:1536], AF.Sigmoid)
                        tg = work_p.tile([128, 512], BF16, tag="tg")
                        nc.scalar.activation(tg[:], ps[:, 1536:2048], AF.Tanh)
                        u = work_p.tile([128, 512], BF16, tag="u")
                        nc.vector.tensor_mul(u[:], sig[:, 0:512], tg[:])
                        cst = c_p.tile([128, 512], BF16, tag="cst")
                        init = 0.0 if c == 0 else clast[g][:, 511:512]
                        nc.vector.tensor_tensor_scan(
                            out=cst[:], data0=sig[:, 512:1024], data1=u[:],
                            initial=init, op0=OP.mult, op1=OP.add,
                        )
                        clast[g] = cst
                        tct = work_p.tile([128, 512], BF16, tag="tc")
                        nc.scalar.activation(tct[:], cst[:], AF.Tanh)
                        nc.vector.tensor_mul(
                            hnew[g][:, 2 + c * 512: 2 + c * 512 + 512],
                            sig[:, 1024:1536], tct[:],
                        )
            return hbufs[K % 2]

        def exchange(ctx, h4, cc_i, cc_o):
            """Masked pair AllReduce of the 4 h-tiles; returns 8 xin tiles
            [fwd-block(4), bwd-block(4)] in local time order."""
            for k in range(4):
                s0 = ex_p.tile([128, L], BF16, tag="exs", bufs=2, name=f"exs0_{k}")
                nc.vector.tensor_scalar_mul(s0[:], h4[k][:, 2:2 + L], met[:, 0:1])
                nc.sync.dma_start(cc_i[0, k * 128:(k + 1) * 128, :], s0[:])
                s1 = ex_p.tile([128, L], BF16, tag="exs", bufs=2, name=f"exs1_{k}")
                nc.vector.tensor_scalar_mul(s1[:], h4[k][:, 2:2 + L], mot[:, 0:1])
                nc.sync.dma_start(cc_i[1, k * 128:(k + 1) * 128, :], s1[:])
            nc.gpsimd.collective_compute(
                "AllReduce", mybir.AluOpType.add, replica_groups=PAIRS,
                ins=[cc_i[:]], outs=[cc_o[:]],
            )
            xs = []
            for blk in range(2):          # 0: fwd block, 1: bwd block
                mown, mother = (met, mot) if blk == 0 else (mot, met)
                for k in range(4):
                    gsl = ex_p.tile([128, L], BF16, tag="exg")
                    nc.sync.dma_start(gsl[:], cc_o[blk, k * 128:(k + 1) * 128, :])
                    xt = xin_p.tile([128, L], BF16, tag="xin")
                    nc.vector.tensor_scalar_mul(xt[:], gsl[:], mown[:, 0:1])
                    nc.vector.scalar_tensor_tensor(
                        out=xt[:], in0=gsl[:, ::-1], scalar=mother[:, 0:1],
                        in1=xt[:], op0=OP.mult, op1=OP.add,
                    )
                    xs.append(xt)
            return xs

        # ---------------- layer 0 ----------------
        with ExitStack() as ctx0:
            xt0 = []
            for k in range(4):
                x = xin_p.tile([128, L], BF16, tag="xin")
                nc.sync.dma_start(x[:], xT_in[k * 128:(k + 1) * 128, :])
                xt0.append(x)
            h0 = run_layer(ctx0, xt0, 4, wih0_in, whh0_in, b0t, K0)
            x1 = exchange(ctx0, h0, cc1_in, cc1_out)

        # ---------------- layer 1 ----------------
        with ExitStack() as ctx1:
            h1 = run_layer(ctx1, x1, 8, wih1_in, whh1_in, b1t, K1)
            x2 = exchange(ctx1, h1, cc2_in, cc2_out)

        # ---------------- feats ----------------
        with ExitStack() as ctxf:
            ps_pool = ctxf.enter_context(tc.tile_pool(name="psf", bufs=2, space="PSUM"))
            wtag = const.tile([128, 8 * T], BF16, tag="wtag")
            nc.sync.dma_start(
                wtag[:].rearrange("p (k m) -> p k m", k=8),
                wtag_in.rearrange("(k p) m -> p k m", p=128))
            for c in range(NCH):
                ps = ps_pool.tile([T, 512], mybir.dt.float32, tag="fps")
                for k in range(8):
                    nc.tensor.matmul(
                        ps[:], wtag[:, k * T:(k + 1) * T],
                        x2[k][:, c * 512:(c + 1) * 512],
                        start=(k == 0), stop=(k == 7),
                    )
                # masked feats: even cores contribute slot0, odd slot1
                fe = ex_p.tile([T, 512], F32, tag="fex", bufs=2, name=f"fe_{c}")
                nc.scalar.activation(fe[:], ps[:], AF.Tanh,
                                     bias=btag_e[:, 0:1], scale=met[0:T, 0:1])
                nc.sync.dma_start(cc3_in[0, :, c * 512:(c + 1) * 512], fe[:])
                fo = ex_p.tile([T, 512], F32, tag="fex", bufs=2, name=f"fo_{c}")
                nc.scalar.activation(fo[:], ps[:], AF.Tanh,
                                     bias=btag_o[:, 0:1], scale=mot[0:T, 0:1])
                nc.sync.dma_start(cc3_in[1, :, c * 512:(c + 1) * 512], fo[:])
            nc.gpsimd.collective_compute(
                "AllReduce", mybir.AluOpType.add, replica_groups=PAIRS,
                ins=[cc3_in[:]], outs=[cc3_out[:]],
            )
            nc.sync.dma_start(FT[:], cc3_out[0])
            nc.sync.dma_start(feats_out[:], FT[:])

        # ---------------- viterbi: fwd deltas + bwd suffix, interleaved ----
        with ExitStack() as ctxv:
            vps = ctxv.enter_context(tc.tile_pool(name="vps", bufs=8, space="PSUM"))
            for j in range(1024):
                for chain in range(2):
                    At = AfTt if chain == 0 else AbTt
                    Dt = Dft if chain == 0 else Dbt
                    init = vft if chain == 0 else vbt
                    fcol = (j - 1) if chain == 0 else (2048 - j)
                    ps = vps.tile([T, T], mybir.dt.float32, tag="vps")
                    prev = init[:, 0:1] if j == 0 else Dt[:, j - 1:j]
                    nc.tensor.matmul(ps[:], prev.to_broadcast([T, T]), identf[:],
                                     is_transpose=True, start=True, stop=False)
                    if j > 0:
                        nc.tensor.matmul(ps[:], FT[:, fcol:fcol + 1].to_broadcast([T, T]),
                                         identf[:], is_transpose=True,
                                         start=False, stop=False)
                    nc.tensor.matmul(ps[:], At[:], identf[:],
                                     is_transpose=True, start=False, stop=True)
                    nc.vector.tensor_reduce(
                        out=Dt[:, j:j + 1], in_=ps[:],
                        axis=mybir.AxisListType.X, op=mybir.AluOpType.max,
                    )
            nc.sync.dma_start(Df_out[:], Dft[:])
            nc.sync.dma_start(Db_out[:], Dbt[:])

    nc.compile()
    return nc


def _prep_inputs(sentence, emb, w_ih0, w_hh0, b_ih0, b_hh0,
                 w_ih1, w_hh1, b_ih1, b_hh1, W_tag, b_tag, A):
    A = np.asarray(A, np.float32)
    variants = []
    for d in range(2):  # 0: fwd, 1: bwd
        sent = np.asarray(sentence)
        sent_loc = sent if d == 0 else sent[::-1]
        X = np.asarray(emb)[sent_loc]                      # [L, E]
        m = {
            "xT": np.ascontiguousarray(X.T).astype(BF),
            "wih0T": np.ascontiguousarray(np.asarray(w_ih0)[d].T).astype(BF),
            "whh0T": np.ascontiguousarray(np.asarray(w_hh0)[d].T).astype(BF),
            "b0": np.ascontiguousarray(
                (np.asarray(b_ih0)[d] + np.asarray(b_hh0)[d]).reshape(16, 128).T
            ).astype(np.float32),
            "wih1T": np.ascontiguousarray(np.asarray(w_ih1)[d].T).astype(BF),
            "whh1T": np.ascontiguousarray(np.asarray(w_hh1)[d].T).astype(BF),
            "b1": np.ascontiguousarray(
                (np.asarray(b_ih1)[d] + np.asarray(b_hh1)[d]).reshape(16, 128).T
            ).astype(np.float32),
            "wtagT": np.ascontiguousarray(np.asarray(W_tag).T).astype(BF),
            "btag_e": (np.asarray(b_tag, np.float32) * (1.0 - d)).reshape(T, 1).copy(),
            "btag_o": (np.asarray(b_tag, np.float32) * float(d)).reshape(T, 1).copy(),
            "AfT": np.ascontiguousarray(A.T),
            "AbT": np.ascontiguousarray(A),
            "vf": np.where(np.arange(T) == 0, 0.0, NEG).astype(np.float32).reshape(T, 1),
            "vb": np.where(np.arange(T) == T - 1, 0.0, NEG).astype(np.float32).reshape(T, 1),
            "me": np.full((128, 1), 1.0 - d, np.float32),
            "mo": np.full((128, 1), float(d), np.float32),
        }
        variants.append(m)
    return [variants[c % 2] for c in range(NCORES)]


def _host_finish(Df_m, Db_m, featsT, A):
    """Backtrace from device delta tables. Df_m/Db_m are m-values (pre-f-add)."""
    A = np.asarray(A, np.float32)
    f = featsT  # [T, L]
    delta1023 = Df_m[:, 1023] + f[:, 1023]
    R1024 = Db_m[:, 1023] + f[:, 1024]
    V = R1024[:, None] + A + delta1023[None, :]
    q, p = np.unravel_index(int(np.argmax(V)), V.shape)
    y = np.zeros(L, np.int64)
    y[1023] = p
    y[1024] = q
    for t in range(1022, -1, -1):
        y[t] = int(np.argmax(Df_m[:, t] + f[:, t] + A[y[t + 1]]))
    for t in range(1025, L):
        Rt = Db_m[:, 2047 - t] + f[:, t]
        y[t] = int(np.argmax(A[:, y[t - 1]] + Rt))
    score = np.float32(V[q, p])
    return y.astype(np.int32), score


def _make_runner(nc):
    """Cached jitted SPMD executor (run_bass_via_pjrt re-jits every call;
    this builds the shard_map jit once and reuses it)."""
    import jax
    import numpy as _np
    import concourse.mybir as mybir
    from concourse import bass2jax
    from jax.experimental.shard_map import shard_map
    from jax.sharding import Mesh, PartitionSpec

    bass2jax.install_neuronx_cc_hook()
    n_cores = NCORES
    partition_name = nc.partition_id_tensor.name if nc.partition_id_tensor else None
    in_names, out_names, out_avals, zero_outs = [], [], [], []
    for alloc in nc.m.functions[0].allocations:
        if not isinstance(alloc, mybir.MemoryLocationSet):
            continue
        name = alloc.memorylocations[0].name
        if alloc.kind == "ExternalInput":
            if name != partition_name:
                in_names.append(name)
        elif alloc.kind == "ExternalOutput":
            shape = tuple(alloc.tensor_shape)
            dtype = mybir.dt.np(alloc.dtype)
            out_names.append(name)
            out_avals.append(jax.core.ShapedArray(shape, dtype))
            zero_outs.append(_np.zeros(shape, dtype))
    n_params = len(in_names)
    n_outs = len(out_avals)
    all_in_names = list(in_names) + list(out_names)
    if partition_name is not None:
        all_in_names.append(partition_name)
    donate = tuple(range(n_params, n_params + n_outs))

    def _body(*args):
        operands = list(args)
        if partition_name is not None:
            operands.append(bass2jax.partition_id_tensor())
        outs = bass2jax._bass_exec_p.bind(
            *operands,
            out_avals=tuple(out_avals),
            in_names=tuple(all_in_names),
            out_names=tuple(out_names),
            lowering_input_output_aliases=(),
            sim_require_finite=True,
            sim_require_nnan=True,
            nc=nc,
        )
        return tuple(outs)

    devices = jax.devices()[:n_cores]
    mesh = Mesh(_np.asarray(devices), ("core",))
    in_specs = (PartitionSpec("core"),) * (n_params + n_outs)
    out_specs = (PartitionSpec("core"),) * n_outs
    sharded = jax.jit(
        shard_map(_body, mesh=mesh, in_specs=in_specs, out_specs=out_specs,
                  check_rep=False),
        donate_argnums=donate, keep_unused=True,
    )

    def run(in_maps):
        concat_in = [
            _np.concatenate([_np.asarray(in_maps[c][nm]) for c in range(n_cores)], axis=0)
            for nm in in_names
        ]
        concat_zeros = [
            _np.zeros((n_cores * z.shape[0], *z.shape[1:]), z.dtype)
            for z in zero_outs
        ]
        out_arrs = sharded(*concat_in, *concat_zeros)
        core0 = {
            nm: _np.asarray(out_arrs[i]).reshape(n_cores, *out_avals[i].shape)[0]
            for i, nm in enumerate(out_names)
        }
        return core0

    return run


def kernel(sentence, emb, w_ih0, w_hh0, b_ih0, b_hh0,
           w_ih1, w_hh1, b_ih1, b_hh1, W_tag, b_tag, A):
    if "nc" not in _CACHE:
        _CACHE["nc"] = _build_program()
        _CACHE["run"] = _make_runner(_CACHE["nc"])

    in_maps = _prep_inputs(sentence, emb, w_ih0, w_hh0, b_ih0, b_hh0,
                           w_ih1, w_hh1, b_ih1, b_hh1, W_tag, b_tag, A)
    r0 = _CACHE["run"](in_maps)
    _CACHE["last_featsT"] = np.asarray(r0["featsT"], np.float32)
    path, score = _host_finish(
        np.asarray(r0["Df"], np.float32),
        np.asarray(r0["Db"], np.float32),
        np.asarray(r0["featsT"], np.float32),
        np.asarray(A, np.float32),
    )
    return path, np.float32(score)
